# revision 15
# baseline (speedup 1.0000x reference)
"""Trainium2 Bass kernel for the Jastrow-factor nn.Module.

Math (per walker w):
  EN: r_en[w,e,n] = |x_we - nuc_n|
      J_en   = sum_{e,n} -q_n * r/(1+softplus(b_en_n)*r)
      J_ennn = s_en * sum_e MLP8(r_en[w,e,:]**2)        (8->32->32->1, silu)
  EE: r_ee[w,p] over 496 unordered pairs p=(i,j)
      J_ee   = sum_p a_p * r/(1+softplus(b_ee)*r)
      J_eenn = s_ee * sum_p MLP1(r_ee[w,p])             (1->32->32->1, silu)
  out[w] = J_en + J_ennn + J_ee + J_eenn

Distribution: pure data parallel, 1024 walkers per core on 8 cores.

The axon tunnel to the device is slow (~20-60 MB/s, RTT tens of ms) and
every byte of input is shipped on every call, so the kernel takes ONE
small fp16 DRAM input per core -- the raw walker coordinates with a
compact constants pack appended as extra columns (~2.1 MB total across
8 cores vs ~29 MB for the naive host-built-weights layout) -- and
derives everything else on device:
  * identity matrix via affine_select, nuclei broadcast via
    partition_broadcast,
  * the EE layer-1 selection matrices via a log-doubled W1_ee row,
    partition_broadcast, and two affine_select band masks
    (weesel[p, f] = W1_ee[f mod 32] iff 0 <= f - 32p < 32),
  * block-diagonal MLP weight matrices via identity matmuls,
  * EN distances computed directly from coordinates in walker-partition
    layout (per-nucleus tensor_scalar ops), then PE-transposed into an
    (electron, nucleus)-partition layout whose MLP needs no segmented
    reduce.

Host-side, kernel.py also enables the JAX persistent compilation cache
and memoizes the BIR json blob, since run_bass_kernel_spmd re-jits a
fresh closure every call (each warm call otherwise re-runs the neuronx
backend compile).

Device layout:
  EN: r2 in [128 walkers, t, (e,n)] -> 16 PE transposes -> r2T
      [(e',n) 128, 2 etile, 1024 walkers]; MLP runs 8 passes (etile x
      4-electron slice) of block-diagonal matmuls; layer-3 and the
      classical charge-weighted term accumulate in one PSUM row [1,1024].
  EE: pair distances via 31 diagonal-offset subtractions in walker-
      partition layout, one big sqrt, PE transposes into 4 tiles
      [124 pairs, 1024 walkers], then the MLP with per-group
      row-selection weight matrices (K=124).
"""

import numpy as np

# Persistent XLA compilation cache: run_bass_kernel_spmd builds a fresh
# jax.jit closure per call, so without this every warm call re-runs the
# neuronx compile (~240ms). The persistent cache is keyed by HLO hash and
# turns that into a disk hit.
import jax

jax.config.update("jax_compilation_cache_dir", "/tmp/jax_comp_cache")
jax.config.update("jax_persistent_cache_min_entry_size_bytes", -1)
jax.config.update("jax_persistent_cache_min_compile_time_secs", 0.0)

N_CORES = 8
N_W, N_E, N_NUC, D_H = 8192, 32, 8, 32
WC = N_W // N_CORES          # walkers per core
NT = WC // 128               # walker tiles per core (8)
P_PAIRS = N_E * (N_E - 1) // 2   # 496
NB = 4                       # rT pair tiles, 124 pairs each
PB = P_PAIRS // NB           # 124
NSEL = PB // 4               # 31 selection matrices
CPK = 264                    # cpak columns


def _pair_list():
    ps = []
    for d in range(1, N_E):
        for e in range(N_E - d):
            ps.append((e, e + d))
    return ps


_PAIRS = _pair_list()
assert len(_PAIRS) == P_PAIRS


def _softplus(x):
    return np.log1p(np.exp(-np.abs(x))) + np.maximum(x, 0.0)


# ----------------------------------------------------------------------------
# device program
# ----------------------------------------------------------------------------

_CACHE = {}


def _build_program():
    from contextlib import ExitStack

    import concourse.bacc as bacc
    import concourse.bass as bass
    import concourse.tile as tile
    from concourse import mybir

    f32 = mybir.dt.float32
    f16 = mybir.dt.float16
    AF = mybir.ActivationFunctionType
    ALU = mybir.AluOpType

    nc = bacc.Bacc()

    XF = NT * 96                 # 768 coord columns
    d_xc = nc.declare_dram_parameter("xc", [128, XF + CPK], f16, isOutput=False)
    d_out = nc.declare_dram_parameter("out", [1, WC], f32, isOutput=True)

    MM = nc.tensor.matmul
    PSUM = bass.MemorySpace.PSUM

    with ExitStack() as top:
        tc = top.enter_context(tile.TileContext(nc))
        const = top.enter_context(tc.tile_pool(name="const", bufs=1))
        work = top.enter_context(tc.tile_pool(name="work", bufs=1))

        xc16 = const.tile([128, XF + CPK], f16, name="xc16", tag="xc16")
        nc.gpsimd.dma_start(out=xc16[:], in_=d_xc[:])
        xwp = const.tile([128, NT, 96], f32, name="xwp", tag="xwp")
        nc.vector.tensor_copy(
            xwp[:], xc16[:, 0:XF].rearrange("p (t f) -> p t f", f=96)
        )
        cpak = const.tile([128, CPK], f32, name="cpak", tag="cpak")
        nc.vector.tensor_copy(cpak[:], xc16[:, XF : XF + CPK])

        wenl3 = cpak[:, 0:1]
        wencls = cpak[:, 1:2]
        b1en = cpak[:, 2:3]
        b2en = cpak[:, 3:4]
        bensp = cpak[:, 4:5]
        weel3 = cpak[:, 5:6]
        b1ee = cpak[:, 6:7]
        b2ee = cpak[:, 7:8]
        beesp = cpak[:, 8:9]
        cconst = cpak[0:1, 13:14]
        w2en_c = cpak[0:32, 16:48]
        w2ee_c = cpak[0:32, 48:80]
        l1bd = cpak[0:32, 80:208]
        nucrow = cpak[0:1, 208:232]
        w1eerow = cpak[0:1, 232:264]

        # ------------------------------------------------------------------
        # on-device constant builds
        # ------------------------------------------------------------------
        ident = const.tile([128, 128], f32, name="ident", tag="ident")
        nc.vector.memset(ident[:], 1.0)
        nc.gpsimd.affine_select(
            out=ident[:],
            in_=ident[:],
            pattern=[[-1, 128]],
            compare_op=ALU.is_equal,
            fill=0.0,
            base=0,
            channel_multiplier=1,
        )
        nucb = const.tile([128, 24], f32, name="nucb", tag="nucb")
        nc.gpsimd.partition_broadcast(nucb[:], nucrow)

        wenl1 = const.tile([128, 128], f32, name="wenl1", tag="wenl1")
        wenl2 = const.tile([128, 128], f32, name="wenl2", tag="wenl2")
        weel2 = const.tile([128, 128], f32, name="weel2", tag="weel2")
        with tc.tile_pool(name="bld", bufs=1, space=PSUM) as bld:
            wl1ps = bld.tile([128, 128], f32, tag="wl1ps")
            for c in range(4):
                MM(
                    wl1ps[32 * c : 32 * c + 32, :],
                    ident[0:32, 0:32],
                    l1bd,
                    start=True,
                    stop=True,
                    tile_position=(0, 32 * c),
                )
            nc.vector.tensor_copy(wenl1[:], wl1ps[:])
            nc.vector.memset(wenl2[:], 0.0)
            nc.vector.memset(weel2[:], 0.0)
            w2ps = bld.tile([128, 128], f32, tag="w2ps")
            e2ps = bld.tile([128, 128], f32, tag="e2ps")
            for g in range(4):
                MM(
                    w2ps[32 * g : 32 * g + 32, 32 * g : 32 * g + 32],
                    ident[0:32, 0:32],
                    w2en_c,
                    start=True,
                    stop=True,
                    tile_position=(0, 32 * g),
                )
                MM(
                    e2ps[32 * g : 32 * g + 32, 32 * g : 32 * g + 32],
                    ident[0:32, 0:32],
                    w2ee_c,
                    start=True,
                    stop=True,
                    tile_position=(0, 32 * g),
                )
            for g in range(4):
                nc.vector.tensor_copy(
                    wenl2[32 * g : 32 * g + 32, 32 * g : 32 * g + 32],
                    w2ps[32 * g : 32 * g + 32, 32 * g : 32 * g + 32],
                )
                nc.vector.tensor_copy(
                    weel2[32 * g : 32 * g + 32, 32 * g : 32 * g + 32],
                    e2ps[32 * g : 32 * g + 32, 32 * g : 32 * g + 32],
                )

        # weesel[p, 128m + col] is W1_ee[0][col mod 32] on the band
        # 0 <= (128m + col) - 32p < 32 and zero elsewhere: build a W1-tiled
        # row by log-doubling, broadcast it to all pair partitions, then
        # mask the band with two affine_selects.
        FW = NSEL * 128
        wrow = const.tile([1, FW], f32, name="wrow", tag="wrow")
        nc.vector.tensor_copy(wrow[0:1, 0:32], w1eerow)
        span = 32
        while span < FW:
            step = min(span, FW - span)
            nc.vector.tensor_copy(
                wrow[0:1, span : span + step], wrow[0:1, 0:step]
            )
            span += step
        weesel = const.tile([PB, NSEL, 128], f32, name="weesel", tag="weesel")
        nc.gpsimd.partition_broadcast(weesel[:], wrow[0:1, :])
        nc.gpsimd.affine_select(
            out=weesel[:],
            in_=weesel[:],
            pattern=[[128, NSEL], [1, 128]],
            compare_op=ALU.is_ge,
            fill=0.0,
            base=0,
            channel_multiplier=-32,
        )
        nc.gpsimd.affine_select(
            out=weesel[:],
            in_=weesel[:],
            pattern=[[-128, NSEL], [-1, 128]],
            compare_op=ALU.is_ge,
            fill=0.0,
            base=31,
            channel_multiplier=32,
        )

        # ------------------------------------------------------------------
        # EE distances in walker-partition layout
        # r2wp[p, t, col] ; col = pair index by diagonal order, padded to 512
        # ------------------------------------------------------------------
        r2wp = work.tile([128, NT, 512], f32)
        nc.vector.memset(r2wp[:], 0.0)
        with tc.tile_pool(name="dpool", bufs=2) as dpool:
            off = 0
            for d in range(1, N_E):
                L = N_E - d
                dd = dpool.tile([128, NT, 96], f32, tag="dd")
                sq = dpool.tile([128, NT, 96], f32, tag="sq")
                nc.vector.tensor_sub(
                    dd[:, :, : 3 * L], xwp[:, :, : 3 * L], xwp[:, :, 3 * d :]
                )
                nc.vector.tensor_mul(
                    sq[:, :, : 3 * L], dd[:, :, : 3 * L], dd[:, :, : 3 * L]
                )
                sq3 = sq[:, :, : 3 * L].rearrange("p t (e c) -> p c t e", c=3)
                nc.vector.tensor_add(
                    r2wp[:, :, off : off + L], sq3[:, 0], sq3[:, 1]
                )
                nc.vector.tensor_add(
                    r2wp[:, :, off : off + L], r2wp[:, :, off : off + L], sq3[:, 2]
                )
                off += L
            assert off == P_PAIRS

        rwp = r2wp
        nc.scalar.sqrt(rwp[:], r2wp[:])

        # ------------------------------------------------------------------
        # EN distances: per-nucleus subtract in walker layout, then
        # transpose to r2T [(e',n) 128, 2 etile, WC]
        # ------------------------------------------------------------------
        r2en_wp = work.tile([128, NT, 32, 8], f32)
        xv = xwp.rearrange("p t (e c) -> p t e c", c=3)
        with tc.tile_pool(name="enpool", bufs=2) as enpool:
            for n in range(8):
                dd = enpool.tile([128, NT, 96], f32, tag="dd")
                sq = enpool.tile([128, NT, 96], f32, tag="sq")
                ddv = dd.rearrange("p t (e c) -> p t e c", c=3)
                for c in range(3):
                    nc.vector.tensor_scalar(
                        ddv[:, :, :, c],
                        xv[:, :, :, c],
                        nucb[:, 3 * n + c : 3 * n + c + 1],
                        None,
                        op0=ALU.add,
                    )
                nc.vector.tensor_mul(sq[:], dd[:], dd[:])
                sq3 = sq.rearrange("p t (e c) -> p t e c", c=3)
                nc.vector.tensor_add(
                    r2en_wp[:, :, :, n], sq3[:, :, :, 0], sq3[:, :, :, 1]
                )
                nc.vector.tensor_add(
                    r2en_wp[:, :, :, n], r2en_wp[:, :, :, n], sq3[:, :, :, 2]
                )

        r2T = work.tile([128, 2, WC], f32)
        with tc.tile_pool(name="trps", bufs=3, space=PSUM) as trps:
            for t in range(NT):
                for et in range(2):
                    pt = trps.tile([128, 128], f32, tag="pt")
                    nc.tensor.transpose(
                        pt[:], r2en_wp[:, t, 16 * et : 16 * et + 16, :], ident[:]
                    )
                    nc.vector.tensor_copy(
                        r2T[:, et, 128 * t : 128 * t + 128], pt[:]
                    )

        ren = work.tile([128, 2, WC], f32)
        nc.scalar.sqrt(ren[:], r2T[:])

        # EN classical: t = r / (1 + softplus(b_en)*r)
        uen = work.tile([128, 2, WC], f32)
        nc.vector.tensor_scalar(
            uen[:], ren[:], bensp, 1.0, op0=ALU.mult, op1=ALU.add
        )
        nc.vector.reciprocal_approx_fast(out=uen[:], in_=uen[:])
        ten = ren
        nc.vector.tensor_mul(ten[:], ren[:], uen[:])

        # ------------------------------------------------------------------
        # EN MLP + classical -> jen_sb [1, WC]
        # ------------------------------------------------------------------
        jen_sb = work.tile([1, WC], f32)
        with (
            tc.tile_pool(name="enps1", bufs=2, space=PSUM) as enps1,
            tc.tile_pool(name="enps2", bufs=1, space=PSUM) as enps2,
            tc.tile_pool(name="jenp", bufs=1, space=PSUM) as jenp,
            tc.tile_pool(name="enh", bufs=2) as enh,
        ):
            jen = jenp.tile([1, WC], f32)
            for et in range(2):
                for h in range(2):
                    MM(
                        jen[0:1, 512 * h : 512 * h + 512],
                        wencls,
                        ten[:, et, 512 * h : 512 * h + 512],
                        start=(et == 0),
                        stop=False,
                        skip_group_check=True,
                    )
            for et in range(2):
                for c in range(4):
                    ps1 = enps1.tile([128, 2, 512], f32, tag="ps1")
                    for h in range(2):
                        MM(
                            ps1[:, h, :],
                            wenl1[32 * c : 32 * c + 32, :],
                            r2T[32 * c : 32 * c + 32, et, 512 * h : 512 * h + 512],
                            start=True,
                            stop=True,
                            tile_position=(32 * c, 0),
                        )
                    h1 = enh.tile([128, 2, 512], f32, tag="h1")
                    nc.scalar.activation(h1[:], ps1[:], AF.Silu, bias=b1en)
                    ps2 = enps2.tile([128, 2, 512], f32, tag="ps2")
                    for h in range(2):
                        MM(ps2[:, h, :], wenl2[:], h1[:, h, :], start=True, stop=True)
                    h2 = enh.tile([128, 2, 512], f32, tag="h2")
                    nc.scalar.activation(h2[:], ps2[:], AF.Silu, bias=b2en)
                    last = et == 1 and c == 3
                    for h in range(2):
                        MM(
                            jen[0:1, 512 * h : 512 * h + 512],
                            wenl3,
                            h2[:, h, :],
                            start=False,
                            stop=last,
                            skip_group_check=True,
                        )
            nc.vector.tensor_copy(jen_sb[:], jen[:])

        # ------------------------------------------------------------------
        # EE transposes: rwp -> rT[b] [124 pairs, 1024 walkers]
        # ------------------------------------------------------------------
        rT = [work.tile([PB, WC], f32, tag=f"rT{b}", name=f"rT{b}") for b in range(NB)]
        with tc.tile_pool(name="ptps", bufs=3, space=PSUM) as ptps:
            for t in range(NT):
                for b in range(NB):
                    pt = ptps.tile([PB, 128], f32, tag="pt")
                    nc.tensor.transpose(
                        pt[:], rwp[:, t, PB * b : PB * b + PB], ident[:]
                    )
                    nc.vector.tensor_copy(rT[b][:, 128 * t : 128 * t + 128], pt[:])

        # ------------------------------------------------------------------
        # EE classical + MLP, accumulating into jee[1, 1024] (PSUM)
        # ------------------------------------------------------------------
        with (
            tc.tile_pool(name="jeeps", bufs=1, space=PSUM) as jeeps,
            tc.tile_pool(name="eecls", bufs=2) as eecls,
        ):
            jee = jeeps.tile([1, WC], f32)
            for b in range(NB):
                u = eecls.tile([PB, WC], f32, tag="u")
                nc.vector.tensor_scalar(
                    u[:], rT[b][:], beesp[0:PB], 1.0, op0=ALU.mult, op1=ALU.add
                )
                nc.vector.reciprocal_approx_fast(out=u[:], in_=u[:])
                t_ee = eecls.tile([PB, WC], f32, tag="t")
                nc.vector.tensor_mul(t_ee[:], rT[b][:], u[:])
                for h in range(2):
                    MM(
                        jee[0:1, 512 * h : 512 * h + 512],
                        cpak[0:PB, 9 + b : 10 + b],
                        t_ee[:, 512 * h : 512 * h + 512],
                        start=(b == 0),
                        stop=False,
                        skip_group_check=True,
                    )

            with (
                tc.tile_pool(name="eeps1", bufs=2, space=PSUM) as eeps1,
                tc.tile_pool(name="eeps2", bufs=1, space=PSUM) as eeps2,
                tc.tile_pool(name="eeh", bufs=2) as eeh,
            ):
                for q in range(PB):
                    b, m = divmod(q, NSEL)
                    ps1 = eeps1.tile([128, 2, 512], f32, tag="ps1")
                    for h in range(2):
                        MM(
                            ps1[:, h, :],
                            weesel[:, m, :],
                            rT[b][:, 512 * h : 512 * h + 512],
                            start=True,
                            stop=True,
                        )
                    h1 = eeh.tile([128, 2, 512], f32, tag="h1")
                    nc.scalar.activation(h1[:], ps1[:], AF.Silu, bias=b1ee)
                    ps2 = eeps2.tile([128, 2, 512], f32, tag="ps2")
                    for h in range(2):
                        MM(ps2[:, h, :], weel2[:], h1[:, h, :], start=True, stop=True)
                    h2 = eeh.tile([128, 2, 512], f32, tag="h2")
                    nc.scalar.activation(h2[:], ps2[:], AF.Silu, bias=b2ee)
                    last = q == PB - 1
                    for h in range(2):
                        MM(
                            jee[0:1, 512 * h : 512 * h + 512],
                            weel3,
                            h2[:, h, :],
                            start=False,
                            stop=last,
                            skip_group_check=True,
                        )

            # final: out = (jee + C) + jen
            out_sb = work.tile([1, WC], f32)
            nc.vector.scalar_tensor_tensor(
                out=out_sb[:],
                in0=jee[:],
                scalar=cconst,
                in1=jen_sb[:],
                op0=ALU.add,
                op1=ALU.add,
            )
            nc.gpsimd.dma_start(out=d_out[:], in_=out_sb[:])

    nc.finalize()
    # The bass->json serialization is pure after finalize; cache it so the
    # per-call HLO lowering doesn't redo the ~60ms conversion.
    blob = nc.to_json_bytes()
    nc.to_json_bytes = lambda: blob
    return nc


def _get_program():
    if "nc" not in _CACHE:
        _CACHE["nc"] = _build_program()
    return _CACHE["nc"]


class _NoTraceResult:
    """Placeholder BassKernelResults for the cached-runner fast path."""

    exec_time_ns = None
    mean_exec_time_ns = None
    instructions_and_trace = None
    profile_json = None


def _make_runner(nc):
    """Build (and warm) a jitted executor equivalent to
    bass2jax.run_bass_via_pjrt's multi-core path.

    run_bass_kernel_spmd rebuilds a fresh jax.jit closure on every call, so
    each warm call pays retrace + lowering + compile-cache read + executable
    load (~40ms). Holding one jit object across calls hits jax's C++ fast
    path instead. The compiled artifact is identical: same bass_exec custom
    call, same NEFF (via the persistent compilation cache), same 8 cores.
    """
    import jax as _jax
    from jax.experimental.shard_map import shard_map
    from jax.sharding import Mesh, PartitionSpec

    from concourse import bass2jax, mybir

    bass2jax.install_neuronx_cc_hook()
    if nc.dbg_addr is not None or nc.partition_id_tensor is not None:
        raise RuntimeError("runner supports plain programs only")

    in_names, out_names, out_avals = [], [], []
    for alloc in nc.m.functions[0].allocations:
        if not isinstance(alloc, mybir.MemoryLocationSet):
            continue
        name = alloc.memorylocations[0].name
        if alloc.kind == "ExternalInput":
            in_names.append(name)
        elif alloc.kind == "ExternalOutput":
            out_names.append(name)
            out_avals.append(
                _jax.core.ShapedArray(
                    tuple(alloc.tensor_shape), mybir.dt.np(alloc.dtype)
                )
            )
    n_params = len(in_names)
    zero_specs = [(tuple(a.shape), a.dtype) for a in out_avals]
    all_names = tuple(in_names) + tuple(out_names)
    donate = tuple(range(n_params, n_params + len(out_names)))

    def _body(*args):
        return tuple(
            bass2jax._bass_exec_p.bind(
                *args,
                out_avals=tuple(out_avals),
                in_names=all_names,
                out_names=tuple(out_names),
                lowering_input_output_aliases=(),
                sim_require_finite=True,
                sim_require_nnan=True,
                nc=nc,
            )
        )

    devices = _jax.devices()[:N_CORES]
    mesh = Mesh(np.asarray(devices), ("core",))
    nin = n_params + len(out_names)
    sharded = _jax.jit(
        shard_map(
            _body,
            mesh=mesh,
            in_specs=(PartitionSpec("core"),) * nin,
            out_specs=(PartitionSpec("core"),) * len(out_names),
            check_rep=False,
        ),
        donate_argnums=donate,
        keep_unused=True,
    )

    def runner(*global_inputs):
        zeros = [
            np.zeros((N_CORES * s[0], *s[1:]), d) for s, d in zero_specs
        ]
        return sharded(*global_inputs, *zeros)

    return runner


# ----------------------------------------------------------------------------
# host-side input prep
# ----------------------------------------------------------------------------


def _shared_inputs(r_nuclei, charges, spin_mask_parallel, b_en, b_ee,
                   W1_en, b1_en, W2_en, b2_en, W3_en, b3_en,
                   W1_ee, b1_ee, W2_ee, b2_ee, W3_ee, b3_ee,
                   scale_en, scale_ee):
    f = np.float32
    nuc = np.asarray(r_nuclei, f)
    q = np.asarray(charges, f)
    sm = np.asarray(spin_mask_parallel)
    s_en = float(np.asarray(scale_en))
    s_ee = float(np.asarray(scale_ee))
    W1e = np.asarray(W1_en, f)
    W3e = np.asarray(W3_en, f)
    W1p = np.asarray(W1_ee, f)
    W3p = np.asarray(W3_ee, f)

    cpak = np.zeros((128, CPK), f)
    cpak[:, 0] = np.tile(s_en * W3e.reshape(32), 4)
    cpak[:, 1] = np.tile(-q, 16)
    cpak[:, 2] = np.tile(np.asarray(b1_en, f).reshape(32), 4)
    cpak[:, 3] = np.tile(np.asarray(b2_en, f).reshape(32), 4)
    cpak[:, 4] = np.tile(_softplus(np.asarray(b_en, f)).reshape(8), 16)
    cpak[:, 5] = np.tile(s_ee * W3p.reshape(32), 4)
    cpak[:, 6] = np.tile(np.asarray(b1_ee, f).reshape(32), 4)
    cpak[:, 7] = np.tile(np.asarray(b2_ee, f).reshape(32), 4)
    cpak[:, 8] = float(_softplus(np.asarray(b_ee, f).reshape(1))[0])
    a_all = np.empty((P_PAIRS,), f)
    for p, (i, j) in enumerate(_PAIRS):
        a_all[p] = 0.25 if sm[i, j] else 0.5
    cpak[0:PB, 9:13] = a_all.reshape(NB, PB).T
    cpak[0, 13] = N_E * s_en * float(np.asarray(b3_en).reshape(-1)[0]) + \
        P_PAIRS * s_ee * float(np.asarray(b3_ee).reshape(-1)[0])
    cpak[0:32, 16:48] = np.asarray(W2_en, f)
    cpak[0:32, 48:80] = np.asarray(W2_ee, f)
    l1bd = np.zeros((32, 128), f)
    for g in range(4):
        l1bd[8 * g : 8 * g + 8, 32 * g : 32 * g + 32] = W1e
    cpak[0:32, 80:208] = l1bd
    cpak[0, 208:232] = (-nuc).reshape(24)
    cpak[0, 232:264] = W1p[0]
    return {"cpak": cpak.astype(np.float16)}


def _run(inputs, trace=False):
    from concourse.bass_utils import run_bass_kernel_spmd

    nc = _get_program()
    shared = _shared_inputs(
        inputs["r_nuclei"], inputs["charges"], inputs["spin_mask_parallel"],
        inputs["b_en"], inputs["b_ee"],
        inputs["W1_en"], inputs["b1_en"], inputs["W2_en"], inputs["b2_en"],
        inputs["W3_en"], inputs["b3_en"],
        inputs["W1_ee"], inputs["b1_ee"], inputs["W2_ee"], inputs["b2_ee"],
        inputs["W3_ee"], inputs["b3_ee"],
        inputs["scale_en"], inputs["scale_ee"],
    )
    r_el = np.asarray(inputs["r_electrons"], np.float16)
    # [core, walker, 96] -> per-core [128, NT*96] walker-partition layout,
    # with the shared constants pack appended as extra columns
    XF = NT * 96
    xc = np.empty((N_CORES, 128, XF + CPK), np.float16)
    xc[:, :, :XF] = r_el.reshape(N_CORES, NT, 128, 96).transpose(0, 2, 1, 3
                                                                 ).reshape(N_CORES, 128, XF)
    xc[:, :, XF:] = shared["cpak"][None]

    if not trace and "runner" in _CACHE:
        try:
            out_arrs = _CACHE["runner"](xc.reshape(N_CORES * 128, XF + CPK))
            out = np.asarray(out_arrs[0]).reshape(-1).astype(np.float32)
            return out, _NoTraceResult()
        except Exception:
            _CACHE.pop("runner", None)

    in_maps = [{"xc": xc[c]} for c in range(N_CORES)]
    res = run_bass_kernel_spmd(nc, in_maps, list(range(N_CORES)), trace=trace)
    out = np.concatenate(
        [np.asarray(r["out"]).reshape(-1) for r in res.results]
    ).astype(np.float32)

    if not trace and "runner" not in _CACHE:
        try:
            runner = _make_runner(nc)
            # warm: jax.jit is lazy, so trigger trace+compile now (persistent
            # cache hit) rather than on the first timed call
            warm = runner(np.zeros((N_CORES * 128, XF + CPK), np.float16))
            np.asarray(warm[0])
            _CACHE["runner"] = runner
        except Exception:
            _CACHE.pop("runner", None)
    return out, res


def kernel(**inputs):
    out, _ = _run(inputs, trace=False)
    return out


# revision 16
# speedup vs baseline: 1.3981x; 1.3981x over previous
"""Trainium2 Bass kernel for the Jastrow-factor nn.Module.

Math (per walker w):
  EN: r_en[w,e,n] = |x_we - nuc_n|
      J_en   = sum_{e,n} -q_n * r/(1+softplus(b_en_n)*r)
      J_ennn = s_en * sum_e MLP8(r_en[w,e,:]**2)        (8->32->32->1, silu)
  EE: r_ee[w,p] over 496 unordered pairs p=(i,j)
      J_ee   = sum_p a_p * r/(1+softplus(b_ee)*r)
      J_eenn = s_ee * sum_p MLP1(r_ee[w,p])             (1->32->32->1, silu)
  out[w] = J_en + J_ennn + J_ee + J_eenn

Distribution: pure data parallel, 1024 walkers per core on 8 cores.

The axon tunnel to the device is slow (~20-60 MB/s, RTT tens of ms) and
every byte of input is shipped on every call, so the kernel takes ONE
small fp16 DRAM input per core -- the raw walker coordinates with a
compact constants pack appended as extra columns (~2.1 MB total across
8 cores vs ~29 MB for the naive host-built-weights layout) -- and
derives everything else on device:
  * identity matrix via affine_select, nuclei broadcast via
    partition_broadcast,
  * the EE layer-1 selection matrices via a log-doubled W1_ee row,
    partition_broadcast, and two affine_select band masks
    (weesel[p, f] = W1_ee[f mod 32] iff 0 <= f - 32p < 32),
  * block-diagonal MLP weight matrices via identity matmuls,
  * EN distances computed directly from coordinates in walker-partition
    layout (per-nucleus tensor_scalar ops), then PE-transposed into an
    (electron, nucleus)-partition layout whose MLP needs no segmented
    reduce.

Host-side, kernel.py also enables the JAX persistent compilation cache
and memoizes the BIR json blob, since run_bass_kernel_spmd re-jits a
fresh closure every call (each warm call otherwise re-runs the neuronx
backend compile).

Device layout:
  EN: r2 in [128 walkers, t, (e,n)] -> 16 PE transposes -> r2T
      [(e',n) 128, 2 etile, 1024 walkers]; MLP runs 8 passes (etile x
      4-electron slice) of block-diagonal matmuls; layer-3 and the
      classical charge-weighted term accumulate in one PSUM row [1,1024].
  EE: pair distances via 31 diagonal-offset subtractions in walker-
      partition layout, one big sqrt, PE transposes into 4 tiles
      [124 pairs, 1024 walkers], then the MLP with per-group
      row-selection weight matrices (K=124).
"""

import numpy as np

# Persistent XLA compilation cache: run_bass_kernel_spmd builds a fresh
# jax.jit closure per call, so without this every warm call re-runs the
# neuronx compile (~240ms). The persistent cache is keyed by HLO hash and
# turns that into a disk hit.
import jax

jax.config.update("jax_compilation_cache_dir", "/tmp/jax_comp_cache")
jax.config.update("jax_persistent_cache_min_entry_size_bytes", -1)
jax.config.update("jax_persistent_cache_min_compile_time_secs", 0.0)

N_CORES = 8
N_W, N_E, N_NUC, D_H = 8192, 32, 8, 32
WC = N_W // N_CORES          # walkers per core
NT = WC // 128               # walker tiles per core (8)
P_PAIRS = N_E * (N_E - 1) // 2   # 496
NB = 4                       # rT pair tiles, 124 pairs each
PB = P_PAIRS // NB           # 124
NSEL = PB // 4               # 31 selection matrices
CPK = 264                    # cpak columns


def _pair_list():
    ps = []
    for d in range(1, N_E):
        for e in range(N_E - d):
            ps.append((e, e + d))
    return ps


_PAIRS = _pair_list()
assert len(_PAIRS) == P_PAIRS


def _softplus(x):
    return np.log1p(np.exp(-np.abs(x))) + np.maximum(x, 0.0)


# ----------------------------------------------------------------------------
# device program
# ----------------------------------------------------------------------------

_CACHE = {}


def _build_program():
    from contextlib import ExitStack

    import concourse.bacc as bacc
    import concourse.bass as bass
    import concourse.tile as tile
    from concourse import mybir

    f32 = mybir.dt.float32
    f16 = mybir.dt.float16
    AF = mybir.ActivationFunctionType
    ALU = mybir.AluOpType

    nc = bacc.Bacc()

    XF = NT * 96                 # 768 coord columns
    d_xc = nc.declare_dram_parameter("xc", [128, XF + CPK], f16, isOutput=False)
    d_out = nc.declare_dram_parameter("out", [1, WC], f32, isOutput=True)

    MM = nc.tensor.matmul
    PSUM = bass.MemorySpace.PSUM

    with ExitStack() as top:
        tc = top.enter_context(tile.TileContext(nc))
        const = top.enter_context(tc.tile_pool(name="const", bufs=1))
        work = top.enter_context(tc.tile_pool(name="work", bufs=1))

        xc16 = const.tile([128, XF + CPK], f16, name="xc16", tag="xc16")
        nc.gpsimd.dma_start(out=xc16[:], in_=d_xc[:])
        xwp = const.tile([128, NT, 96], f32, name="xwp", tag="xwp")
        nc.vector.tensor_copy(
            xwp[:], xc16[:, 0:XF].rearrange("p (t f) -> p t f", f=96)
        )
        cpak = const.tile([128, CPK], f32, name="cpak", tag="cpak")
        nc.vector.tensor_copy(cpak[:], xc16[:, XF : XF + CPK])

        wenl3 = cpak[:, 0:1]
        wencls = cpak[:, 1:2]
        b1en = cpak[:, 2:3]
        b2en = cpak[:, 3:4]
        bensp = cpak[:, 4:5]
        weel3 = cpak[:, 5:6]
        b1ee = cpak[:, 6:7]
        b2ee = cpak[:, 7:8]
        beesp = cpak[:, 8:9]
        cconst = cpak[0:1, 13:14]
        w2en_c = cpak[0:32, 16:48]
        w2ee_c = cpak[0:32, 48:80]
        l1bd = cpak[0:32, 80:208]
        nucrow = cpak[0:1, 208:232]
        w1eerow = cpak[0:1, 232:264]

        # ------------------------------------------------------------------
        # on-device constant builds
        # ------------------------------------------------------------------
        ident = const.tile([128, 128], f32, name="ident", tag="ident")
        nc.vector.memset(ident[:], 1.0)
        nc.gpsimd.affine_select(
            out=ident[:],
            in_=ident[:],
            pattern=[[-1, 128]],
            compare_op=ALU.is_equal,
            fill=0.0,
            base=0,
            channel_multiplier=1,
        )
        nucb = const.tile([128, 24], f32, name="nucb", tag="nucb")
        nc.gpsimd.partition_broadcast(nucb[:], nucrow)

        wenl1 = const.tile([128, 128], f32, name="wenl1", tag="wenl1")
        wenl2 = const.tile([128, 128], f32, name="wenl2", tag="wenl2")
        weel2 = const.tile([128, 128], f32, name="weel2", tag="weel2")
        with tc.tile_pool(name="bld", bufs=1, space=PSUM) as bld:
            wl1ps = bld.tile([128, 128], f32, tag="wl1ps")
            for c in range(4):
                MM(
                    wl1ps[32 * c : 32 * c + 32, :],
                    ident[0:32, 0:32],
                    l1bd,
                    start=True,
                    stop=True,
                    tile_position=(0, 32 * c),
                )
            nc.vector.tensor_copy(wenl1[:], wl1ps[:])
            nc.vector.memset(wenl2[:], 0.0)
            nc.vector.memset(weel2[:], 0.0)
            w2ps = bld.tile([128, 128], f32, tag="w2ps")
            e2ps = bld.tile([128, 128], f32, tag="e2ps")
            for g in range(4):
                MM(
                    w2ps[32 * g : 32 * g + 32, 32 * g : 32 * g + 32],
                    ident[0:32, 0:32],
                    w2en_c,
                    start=True,
                    stop=True,
                    tile_position=(0, 32 * g),
                )
                MM(
                    e2ps[32 * g : 32 * g + 32, 32 * g : 32 * g + 32],
                    ident[0:32, 0:32],
                    w2ee_c,
                    start=True,
                    stop=True,
                    tile_position=(0, 32 * g),
                )
            for g in range(4):
                nc.vector.tensor_copy(
                    wenl2[32 * g : 32 * g + 32, 32 * g : 32 * g + 32],
                    w2ps[32 * g : 32 * g + 32, 32 * g : 32 * g + 32],
                )
                nc.vector.tensor_copy(
                    weel2[32 * g : 32 * g + 32, 32 * g : 32 * g + 32],
                    e2ps[32 * g : 32 * g + 32, 32 * g : 32 * g + 32],
                )

        # weesel[p, 128m + col] is W1_ee[0][col mod 32] on the band
        # 0 <= (128m + col) - 32p < 32 and zero elsewhere: build a W1-tiled
        # row by log-doubling, broadcast it to all pair partitions, then
        # mask the band with two affine_selects.
        FW = NSEL * 128
        wrow = const.tile([1, FW], f32, name="wrow", tag="wrow")
        nc.vector.tensor_copy(wrow[0:1, 0:32], w1eerow)
        span = 32
        while span < FW:
            step = min(span, FW - span)
            nc.vector.tensor_copy(
                wrow[0:1, span : span + step], wrow[0:1, 0:step]
            )
            span += step
        weesel = const.tile([PB, NSEL, 128], f32, name="weesel", tag="weesel")
        nc.gpsimd.partition_broadcast(weesel[:], wrow[0:1, :])
        nc.gpsimd.affine_select(
            out=weesel[:],
            in_=weesel[:],
            pattern=[[128, NSEL], [1, 128]],
            compare_op=ALU.is_ge,
            fill=0.0,
            base=0,
            channel_multiplier=-32,
        )
        nc.gpsimd.affine_select(
            out=weesel[:],
            in_=weesel[:],
            pattern=[[-128, NSEL], [-1, 128]],
            compare_op=ALU.is_ge,
            fill=0.0,
            base=31,
            channel_multiplier=32,
        )

        # ------------------------------------------------------------------
        # EE distances in walker-partition layout
        # r2wp[p, t, col] ; col = pair index by diagonal order, padded to 512
        # ------------------------------------------------------------------
        r2wp = work.tile([128, NT, 512], f32)
        nc.vector.memset(r2wp[:], 0.0)
        with tc.tile_pool(name="dpool", bufs=2) as dpool:
            off = 0
            for d in range(1, N_E):
                L = N_E - d
                dd = dpool.tile([128, NT, 96], f32, tag="dd")
                sq = dpool.tile([128, NT, 96], f32, tag="sq")
                nc.vector.tensor_sub(
                    dd[:, :, : 3 * L], xwp[:, :, : 3 * L], xwp[:, :, 3 * d :]
                )
                nc.vector.tensor_mul(
                    sq[:, :, : 3 * L], dd[:, :, : 3 * L], dd[:, :, : 3 * L]
                )
                sq3 = sq[:, :, : 3 * L].rearrange("p t (e c) -> p c t e", c=3)
                nc.vector.tensor_add(
                    r2wp[:, :, off : off + L], sq3[:, 0], sq3[:, 1]
                )
                nc.vector.tensor_add(
                    r2wp[:, :, off : off + L], r2wp[:, :, off : off + L], sq3[:, 2]
                )
                off += L
            assert off == P_PAIRS

        rwp = r2wp
        nc.scalar.sqrt(rwp[:], r2wp[:])

        # ------------------------------------------------------------------
        # EN distances: per-nucleus subtract in walker layout, then
        # transpose to r2T [(e',n) 128, 2 etile, WC]
        # ------------------------------------------------------------------
        r2en_wp = work.tile([128, NT, 32, 8], f32)
        xv = xwp.rearrange("p t (e c) -> p t e c", c=3)
        with tc.tile_pool(name="enpool", bufs=2) as enpool:
            for n in range(8):
                dd = enpool.tile([128, NT, 96], f32, tag="dd")
                sq = enpool.tile([128, NT, 96], f32, tag="sq")
                ddv = dd.rearrange("p t (e c) -> p t e c", c=3)
                for c in range(3):
                    nc.vector.tensor_scalar(
                        ddv[:, :, :, c],
                        xv[:, :, :, c],
                        nucb[:, 3 * n + c : 3 * n + c + 1],
                        None,
                        op0=ALU.add,
                    )
                nc.vector.tensor_mul(sq[:], dd[:], dd[:])
                sq3 = sq.rearrange("p t (e c) -> p t e c", c=3)
                nc.vector.tensor_add(
                    r2en_wp[:, :, :, n], sq3[:, :, :, 0], sq3[:, :, :, 1]
                )
                nc.vector.tensor_add(
                    r2en_wp[:, :, :, n], r2en_wp[:, :, :, n], sq3[:, :, :, 2]
                )

        r2T = work.tile([128, 2, WC], f32)
        with tc.tile_pool(name="trps", bufs=3, space=PSUM) as trps:
            for t in range(NT):
                for et in range(2):
                    pt = trps.tile([128, 128], f32, tag="pt")
                    nc.tensor.transpose(
                        pt[:], r2en_wp[:, t, 16 * et : 16 * et + 16, :], ident[:]
                    )
                    nc.vector.tensor_copy(
                        r2T[:, et, 128 * t : 128 * t + 128], pt[:]
                    )

        ren = work.tile([128, 2, WC], f32)
        nc.scalar.sqrt(ren[:], r2T[:])

        # EN classical: t = r / (1 + softplus(b_en)*r)
        uen = work.tile([128, 2, WC], f32)
        nc.vector.tensor_scalar(
            uen[:], ren[:], bensp, 1.0, op0=ALU.mult, op1=ALU.add
        )
        nc.vector.reciprocal_approx_fast(out=uen[:], in_=uen[:])
        ten = ren
        nc.vector.tensor_mul(ten[:], ren[:], uen[:])

        # ------------------------------------------------------------------
        # EN MLP + classical -> jen_sb [1, WC]
        # ------------------------------------------------------------------
        jen_sb = work.tile([1, WC], f32)
        with (
            tc.tile_pool(name="enps1", bufs=2, space=PSUM) as enps1,
            tc.tile_pool(name="enps2", bufs=1, space=PSUM) as enps2,
            tc.tile_pool(name="jenp", bufs=1, space=PSUM) as jenp,
            tc.tile_pool(name="enh", bufs=2) as enh,
        ):
            jen = jenp.tile([1, WC], f32)
            for et in range(2):
                for h in range(2):
                    MM(
                        jen[0:1, 512 * h : 512 * h + 512],
                        wencls,
                        ten[:, et, 512 * h : 512 * h + 512],
                        start=(et == 0),
                        stop=False,
                        skip_group_check=True,
                    )
            for et in range(2):
                for c in range(4):
                    ps1 = enps1.tile([128, 2, 512], f32, tag="ps1")
                    for h in range(2):
                        MM(
                            ps1[:, h, :],
                            wenl1[32 * c : 32 * c + 32, :],
                            r2T[32 * c : 32 * c + 32, et, 512 * h : 512 * h + 512],
                            start=True,
                            stop=True,
                            tile_position=(32 * c, 0),
                        )
                    h1 = enh.tile([128, 2, 512], f32, tag="h1")
                    nc.scalar.activation(h1[:], ps1[:], AF.Silu, bias=b1en)
                    ps2 = enps2.tile([128, 2, 512], f32, tag="ps2")
                    for h in range(2):
                        MM(ps2[:, h, :], wenl2[:], h1[:, h, :], start=True, stop=True)
                    h2 = enh.tile([128, 2, 512], f32, tag="h2")
                    nc.scalar.activation(h2[:], ps2[:], AF.Silu, bias=b2en)
                    last = et == 1 and c == 3
                    for h in range(2):
                        MM(
                            jen[0:1, 512 * h : 512 * h + 512],
                            wenl3,
                            h2[:, h, :],
                            start=False,
                            stop=last,
                            skip_group_check=True,
                        )
            nc.vector.tensor_copy(jen_sb[:], jen[:])

        # ------------------------------------------------------------------
        # EE transposes: rwp -> rT[b] [124 pairs, 1024 walkers]
        # ------------------------------------------------------------------
        rT = [work.tile([PB, WC], f32, tag=f"rT{b}", name=f"rT{b}") for b in range(NB)]
        with tc.tile_pool(name="ptps", bufs=3, space=PSUM) as ptps:
            for t in range(NT):
                for b in range(NB):
                    pt = ptps.tile([PB, 128], f32, tag="pt")
                    nc.tensor.transpose(
                        pt[:], rwp[:, t, PB * b : PB * b + PB], ident[:]
                    )
                    nc.vector.tensor_copy(rT[b][:, 128 * t : 128 * t + 128], pt[:])

        # ------------------------------------------------------------------
        # EE classical + MLP, accumulating into jee[1, 1024] (PSUM)
        # ------------------------------------------------------------------
        with (
            tc.tile_pool(name="jeeps", bufs=1, space=PSUM) as jeeps,
            tc.tile_pool(name="eecls", bufs=2) as eecls,
        ):
            jee = jeeps.tile([1, WC], f32)
            for b in range(NB):
                u = eecls.tile([PB, WC], f32, tag="u")
                nc.vector.tensor_scalar(
                    u[:], rT[b][:], beesp[0:PB], 1.0, op0=ALU.mult, op1=ALU.add
                )
                nc.vector.reciprocal_approx_fast(out=u[:], in_=u[:])
                t_ee = eecls.tile([PB, WC], f32, tag="t")
                nc.vector.tensor_mul(t_ee[:], rT[b][:], u[:])
                for h in range(2):
                    MM(
                        jee[0:1, 512 * h : 512 * h + 512],
                        cpak[0:PB, 9 + b : 10 + b],
                        t_ee[:, 512 * h : 512 * h + 512],
                        start=(b == 0),
                        stop=False,
                        skip_group_check=True,
                    )

            with (
                tc.tile_pool(name="eeps1", bufs=2, space=PSUM) as eeps1,
                tc.tile_pool(name="eeps2", bufs=1, space=PSUM) as eeps2,
                tc.tile_pool(name="eeh", bufs=2) as eeh,
            ):
                for q in range(PB):
                    b, m = divmod(q, NSEL)
                    ps1 = eeps1.tile([128, 2, 512], f32, tag="ps1")
                    for h in range(2):
                        MM(
                            ps1[:, h, :],
                            weesel[:, m, :],
                            rT[b][:, 512 * h : 512 * h + 512],
                            start=True,
                            stop=True,
                        )
                    h1 = eeh.tile([128, 2, 512], f32, tag="h1")
                    nc.scalar.activation(h1[:], ps1[:], AF.Silu, bias=b1ee)
                    ps2 = eeps2.tile([128, 2, 512], f32, tag="ps2")
                    for h in range(2):
                        MM(ps2[:, h, :], weel2[:], h1[:, h, :], start=True, stop=True)
                    h2 = eeh.tile([128, 2, 512], f32, tag="h2")
                    nc.scalar.activation(h2[:], ps2[:], AF.Silu, bias=b2ee)
                    last = q == PB - 1
                    for h in range(2):
                        MM(
                            jee[0:1, 512 * h : 512 * h + 512],
                            weel3,
                            h2[:, h, :],
                            start=False,
                            stop=last,
                            skip_group_check=True,
                        )

            # final: out = (jee + C) + jen
            out_sb = work.tile([1, WC], f32)
            nc.vector.scalar_tensor_tensor(
                out=out_sb[:],
                in0=jee[:],
                scalar=cconst,
                in1=jen_sb[:],
                op0=ALU.add,
                op1=ALU.add,
            )
            nc.gpsimd.dma_start(out=d_out[:], in_=out_sb[:])

    nc.finalize()
    # The bass->json serialization is pure after finalize; cache it so the
    # per-call HLO lowering doesn't redo the ~60ms conversion.
    blob = nc.to_json_bytes()
    nc.to_json_bytes = lambda: blob
    return nc


def _get_program():
    if "nc" not in _CACHE:
        _CACHE["nc"] = _build_program()
    return _CACHE["nc"]


class _NoTraceResult:
    """Placeholder BassKernelResults for the cached-runner fast path."""

    exec_time_ns = None
    mean_exec_time_ns = None
    instructions_and_trace = None
    profile_json = None


def _make_runner(nc):
    """Build (and warm) a jitted executor equivalent to
    bass2jax.run_bass_via_pjrt's multi-core path.

    run_bass_kernel_spmd rebuilds a fresh jax.jit closure on every call, so
    each warm call pays retrace + lowering + compile-cache read + executable
    load (~40ms). Holding one jit object across calls hits jax's C++ fast
    path instead. The compiled artifact is identical: same bass_exec custom
    call, same NEFF (via the persistent compilation cache), same 8 cores.
    """
    import jax as _jax
    from jax.experimental.shard_map import shard_map
    from jax.sharding import Mesh, PartitionSpec

    from concourse import bass2jax, mybir

    bass2jax.install_neuronx_cc_hook()
    if nc.dbg_addr is not None:
        raise RuntimeError("runner does not support debug programs")
    partition_name = (
        nc.partition_id_tensor.name if nc.partition_id_tensor else None
    )

    in_names, out_names, out_avals = [], [], []
    for alloc in nc.m.functions[0].allocations:
        if not isinstance(alloc, mybir.MemoryLocationSet):
            continue
        name = alloc.memorylocations[0].name
        if alloc.kind == "ExternalInput":
            if name != partition_name:
                in_names.append(name)
        elif alloc.kind == "ExternalOutput":
            out_names.append(name)
            out_avals.append(
                _jax.core.ShapedArray(
                    tuple(alloc.tensor_shape), mybir.dt.np(alloc.dtype)
                )
            )
    n_params = len(in_names)
    zero_specs = [(tuple(a.shape), a.dtype) for a in out_avals]
    all_names = tuple(in_names) + tuple(out_names)
    if partition_name is not None:
        all_names = all_names + (partition_name,)
    donate = tuple(range(n_params, n_params + len(out_names)))

    def _body(*args):
        operands = list(args)
        if partition_name is not None:
            operands.append(bass2jax.partition_id_tensor())
        return tuple(
            bass2jax._bass_exec_p.bind(
                *operands,
                out_avals=tuple(out_avals),
                in_names=all_names,
                out_names=tuple(out_names),
                lowering_input_output_aliases=(),
                sim_require_finite=True,
                sim_require_nnan=True,
                nc=nc,
            )
        )

    devices = _jax.devices()[:N_CORES]
    mesh = Mesh(np.asarray(devices), ("core",))
    nin = n_params + len(out_names)
    sharded = _jax.jit(
        shard_map(
            _body,
            mesh=mesh,
            in_specs=(PartitionSpec("core"),) * nin,
            out_specs=(PartitionSpec("core"),) * len(out_names),
            check_rep=False,
        ),
        donate_argnums=donate,
        keep_unused=True,
    )

    def runner(*global_inputs):
        zeros = [
            np.zeros((N_CORES * s[0], *s[1:]), d) for s, d in zero_specs
        ]
        return sharded(*global_inputs, *zeros)

    return runner


# ----------------------------------------------------------------------------
# host-side input prep
# ----------------------------------------------------------------------------


def _shared_inputs(r_nuclei, charges, spin_mask_parallel, b_en, b_ee,
                   W1_en, b1_en, W2_en, b2_en, W3_en, b3_en,
                   W1_ee, b1_ee, W2_ee, b2_ee, W3_ee, b3_ee,
                   scale_en, scale_ee):
    f = np.float32
    nuc = np.asarray(r_nuclei, f)
    q = np.asarray(charges, f)
    sm = np.asarray(spin_mask_parallel)
    s_en = float(np.asarray(scale_en))
    s_ee = float(np.asarray(scale_ee))
    W1e = np.asarray(W1_en, f)
    W3e = np.asarray(W3_en, f)
    W1p = np.asarray(W1_ee, f)
    W3p = np.asarray(W3_ee, f)

    cpak = np.zeros((128, CPK), f)
    cpak[:, 0] = np.tile(s_en * W3e.reshape(32), 4)
    cpak[:, 1] = np.tile(-q, 16)
    cpak[:, 2] = np.tile(np.asarray(b1_en, f).reshape(32), 4)
    cpak[:, 3] = np.tile(np.asarray(b2_en, f).reshape(32), 4)
    cpak[:, 4] = np.tile(_softplus(np.asarray(b_en, f)).reshape(8), 16)
    cpak[:, 5] = np.tile(s_ee * W3p.reshape(32), 4)
    cpak[:, 6] = np.tile(np.asarray(b1_ee, f).reshape(32), 4)
    cpak[:, 7] = np.tile(np.asarray(b2_ee, f).reshape(32), 4)
    cpak[:, 8] = float(_softplus(np.asarray(b_ee, f).reshape(1))[0])
    a_all = np.empty((P_PAIRS,), f)
    for p, (i, j) in enumerate(_PAIRS):
        a_all[p] = 0.25 if sm[i, j] else 0.5
    cpak[0:PB, 9:13] = a_all.reshape(NB, PB).T
    cpak[0, 13] = N_E * s_en * float(np.asarray(b3_en).reshape(-1)[0]) + \
        P_PAIRS * s_ee * float(np.asarray(b3_ee).reshape(-1)[0])
    cpak[0:32, 16:48] = np.asarray(W2_en, f)
    cpak[0:32, 48:80] = np.asarray(W2_ee, f)
    l1bd = np.zeros((32, 128), f)
    for g in range(4):
        l1bd[8 * g : 8 * g + 8, 32 * g : 32 * g + 32] = W1e
    cpak[0:32, 80:208] = l1bd
    cpak[0, 208:232] = (-nuc).reshape(24)
    cpak[0, 232:264] = W1p[0]
    return {"cpak": cpak.astype(np.float16)}


def _run(inputs, trace=False):
    from concourse.bass_utils import run_bass_kernel_spmd

    nc = _get_program()
    shared = _shared_inputs(
        inputs["r_nuclei"], inputs["charges"], inputs["spin_mask_parallel"],
        inputs["b_en"], inputs["b_ee"],
        inputs["W1_en"], inputs["b1_en"], inputs["W2_en"], inputs["b2_en"],
        inputs["W3_en"], inputs["b3_en"],
        inputs["W1_ee"], inputs["b1_ee"], inputs["W2_ee"], inputs["b2_ee"],
        inputs["W3_ee"], inputs["b3_ee"],
        inputs["scale_en"], inputs["scale_ee"],
    )
    r_el = np.asarray(inputs["r_electrons"], np.float16)
    # [core, walker, 96] -> per-core [128, NT*96] walker-partition layout,
    # with the shared constants pack appended as extra columns
    XF = NT * 96
    xc = np.empty((N_CORES, 128, XF + CPK), np.float16)
    xc[:, :, :XF] = r_el.reshape(N_CORES, NT, 128, 96).transpose(0, 2, 1, 3
                                                                 ).reshape(N_CORES, 128, XF)
    xc[:, :, XF:] = shared["cpak"][None]

    if not trace and "runner" in _CACHE:
        try:
            out_arrs = _CACHE["runner"](xc.reshape(N_CORES * 128, XF + CPK))
            out = np.asarray(out_arrs[0]).reshape(-1).astype(np.float32)
            return out, _NoTraceResult()
        except Exception:
            _CACHE.pop("runner", None)

    in_maps = [{"xc": xc[c]} for c in range(N_CORES)]
    res = run_bass_kernel_spmd(nc, in_maps, list(range(N_CORES)), trace=trace)
    out = np.concatenate(
        [np.asarray(r["out"]).reshape(-1) for r in res.results]
    ).astype(np.float32)

    if not trace and "runner" not in _CACHE:
        try:
            runner = _make_runner(nc)
            # warm: jax.jit is lazy, so trigger trace+compile now (persistent
            # cache hit) rather than on the first timed call
            warm = runner(np.zeros((N_CORES * 128, XF + CPK), np.float16))
            np.asarray(warm[0])
            _CACHE["runner"] = runner
        except Exception:
            _CACHE.pop("runner", None)
    return out, res


def kernel(**inputs):
    out, _ = _run(inputs, trace=False)
    return out
